# revision 28
# baseline (speedup 1.0000x reference)
"""Blockwise linear fusion kernel for Trainium2 (8 NeuronCores).

Computes out[b,c,h,w] = sum_k x[b,k,c,h,w] * weights[h//16, w//16, c, k]
  x: (4, 32, 3, 512, 512) f32, weights: (32, 32, 3, 32) f32 -> out: (4, 3, 512, 512) f32

Strategy:
 - Shard H across the 8 cores: each core handles 64 rows = 4 row-blocks.
 - On each core, the K=32 weighted reduction runs on TensorE as block-diagonal
   matmuls: SBUF x-tiles are laid out [partition=(b,k8,i), free=(r16,w256)],
   and for each output 16x16 block a matmul with a [128,16] block-diagonal
   weight tile contracts k within 16 (b,i) groups -> out[16, 256] in PSUM,
   accumulated over 4 k-chunks.
 - The host pre-transposes each core's x slice (cast to fp16) into the exact
   tile layout so every x DMA is a flat contiguous [128, 8192] transfer, and
   pre-expands the weights into the block-diagonal SBUF layout.
 - Input tiles stream on the Sync HWDGE ring; weight/output DMAs ride the
   Scalar ring so their semaphore waits never stall the input stream.
"""

import sys

sys.path.insert(0, "/opt/trn_rl_repo")

import numpy as np

import concourse.bass as bass  # noqa: F401
import concourse.mybir as mybir
import concourse.tile as tile
from concourse import bacc
from concourse.bass_utils import run_bass_kernel_spmd

# Problem constants (hardcoded per harness contract)
B, K, C, H, W = 4, 32, 3, 512, 512
BS = 16
NCORES = 8
HD = H // NCORES  # 64 rows per core
IB = HD // BS  # 4 i-blocks per core
JB = W // BS  # 32 j-blocks
KC = 4  # number of k-chunks
KCS = K // KC  # 8 k per chunk
G = B * IB  # 16 groups (b, i)
WHALF = W // 2  # 256
JH = JB // 2  # 16 j's per w-half
TFREE = BS * WHALF  # 4096 free elements per (kc, w-half) chunk

_DT = mybir.dt.float16  # matmul input dtype (full-rate PE, half DMA traffic)
_NPDT = np.float16
_F32 = mybir.dt.float32

_CACHE = {}


class _FastEndTileContext(tile.TileContext):
    """TileContext with a cheaper epilogue: the stock one runs two full
    EVSEM butterfly barriers (~1.4us/hop via the DMA queue); sem-only
    barriers skip the per-engine InstDrains."""

    def _drain_and_barrier(self, tick_clock, wait_clock):
        from concourse.vector_clock import ScopedClock

        drain_inst = self.nc.sync.drain()
        wait_clock.add_sem_waits(
            drain_inst.ins, ScopedClock({None: tick_clock.global_clock})
        )
        self.nc.all_engine_barrier(sem_only=True)
        popped = self.nc._tile_sem_poison_stack.pop()
        assert popped is self._sem_poison
        self.nc.clear_and_free_semaphores(list(self.sems.allocated().values()))
        self.nc.all_engine_barrier(sem_only=True)


def _build_program():
    nc = bacc.Bacc(
        "TRN2",
        target_bir_lowering=False,
        debug=False,
        num_devices=NCORES,
        enable_partition_id=False,
    )

    # x pre-arranged on host: [c, wh, kcp, partition=(b,kk,i), free=(kc2,r,w')]
    x_d = nc.dram_tensor("x", [C, 2, KC // 2, 128, 2 * TFREE], _DT, kind="ExternalInput").ap()
    # compact weights: [32=(kk,i), (c,kc,j)=384, i'=4]; expanded on-device
    wb_d = nc.dram_tensor("wb", [32, C * KC * JB * IB], _DT, kind="ExternalInput").ap()
    # out in staging layout: [partition=(b,i), free=(c,r,w)]; host un-permutes
    out_d = nc.dram_tensor("out", [G, C * BS * W], _DT, kind="ExternalOutput").ap()
    outv = out_d.rearrange("g (c r w) -> g c r w", c=C, r=BS)

    with _FastEndTileContext(nc) as tc:
        with (
            tc.tile_pool(name="wpool", bufs=1) as wpool,
            tc.tile_pool(name="xpool", bufs=6) as xpool,
            tc.tile_pool(name="opool", bufs=3) as opool,
            tc.tile_pool(name="ppool", bufs=8, space="PSUM") as ppool,
        ):
            # expand the compact blob to the block-diagonal layout: zero-fill,
            # then 4 strided DMAs place each batch's diagonal block
            wsb = wpool.tile([128, C * KC * JB * G], _DT)
            nc.vector.memset(wsb[:], 0.0)
            wsbv = wsb[:].rearrange("p (t g) -> p t g", g=G)
            wbv = wb_d.rearrange("p (t i) -> p t i", i=IB)
            for b in range(B):
                nc.gpsimd.dma_start(
                    wsbv[b * 32 : (b + 1) * 32, :, b * IB : (b + 1) * IB], wbv
                )

            for c in range(C):
                for wh in range(2):
                    last_round = c == C - 1 and wh == 1
                    # per-round output staging: [16=(b,i), free=(r,jl,q)]
                    osb = opool.tile([G, BS * JH * BS], _DT)
                    osbv = osb[:].rearrange("g (r j q) -> g j r q", r=BS, j=JH)
                    banks = [
                        ppool.tile([G, 512], _F32, name="bank", tag="bank")
                        for _ in range(8)
                    ]
                    for kcp in range(KC // 2):
                        if not last_round:
                            # one 2MB transfer covering two k-chunks
                            xt = xpool.tile([128, 2 * TFREE], _DT)
                            ring = (
                                nc.sync
                                if ((c * 2 + wh) * 2 + kcp) % 2 == 0
                                else nc.scalar
                            )
                            ring.dma_start(xt[:], x_d[c, wh, kcp])
                            xv = xt[:].rearrange(
                                "p (k2 r w) -> p k2 r w", k2=2, r=BS
                            )
                            subviews = [xv[:, 0], xv[:, 1]]
                        else:
                            # kc-granular 1MB transfers: later matmuls start
                            # sooner, shortening the kernel tail
                            subviews = []
                            for kc2 in range(2):
                                xts = xpool.tile(
                                    [128, TFREE], _DT, name="xts", tag="xts", bufs=4
                                )
                                ring = nc.sync if kc2 == 0 else nc.scalar
                                ring.dma_start(
                                    xts[:],
                                    x_d[c, wh, kcp][
                                        :, kc2 * TFREE : (kc2 + 1) * TFREE
                                    ],
                                )
                                subviews.append(
                                    xts[:].rearrange("p (r w) -> p r w", r=BS)
                                )
                        for kc2 in range(2):
                            kc = kcp * 2 + kc2
                            xvk = subviews[kc2]
                            for jl in range(JH):
                                j = wh * JH + jl  # global j block
                                m = jl // 2  # bank index
                                half = jl % 2
                                col0 = ((c * KC + kc) * JB + j) * G
                                nc.tensor.matmul(
                                    banks[m][:, half * 256 : half * 256 + 256],
                                    wsb[:, col0 : col0 + G],
                                    xvk[:, :, jl * BS : (jl + 1) * BS],
                                    start=(kc == 0 and half == 0),
                                    stop=(kc == KC - 1 and half == 1),
                                )
                    # evacuate psum -> osb; stream out on the SWDGE queue so
                    # store waits never stall the input rings
                    for m in range(8):
                        srcv = banks[m][:].rearrange(
                            "g (jj r q) -> g jj r q", jj=2, r=BS, q=BS
                        )
                        nc.vector.tensor_copy(osbv[:, 2 * m : 2 * m + 2, :, :], srcv)
                        if last_round and m % 2 == 1:
                            # pair-granular stores shorten the kernel tail
                            w0 = wh * WHALF + (m - 1) * 2 * BS
                            ow = outv[:, c, :, w0 : w0 + 4 * BS]
                            osl = osb[:].rearrange("g (r w) -> g r w", r=BS)[
                                :, :, (m - 1) * 2 * BS : (m + 1) * 2 * BS
                            ]
                            nc.gpsimd.dma_start(ow, osl)
                    if not last_round:
                        ow = outv[:, c, :, wh * WHALF : (wh + 1) * WHALF]
                        osl = osb[:].rearrange("g (r w) -> g r w", r=BS)
                        nc.gpsimd.dma_start(ow, osl)

    nc.compile()
    return nc


def _host_arrange_x(x_dev):
    """(B, K, C, HD, W) -> [C, 2, KC/2, 128, 2*TFREE] fp16 tile layout.

    partition p = b*(KCS*IB) + kk*IB + i ; free f = kc2*TFREE + r*WHALF + w'
    """
    t = x_dev.reshape(B, KC // 2, 2, KCS, C, IB, BS, 2, WHALF)
    # -> c, wh, kcp, b, kk, i, kc2, r, w'
    t = t.transpose(4, 7, 1, 0, 3, 5, 2, 6, 8)
    return t.astype(_NPDT).reshape(C, 2, KC // 2, 128, 2 * TFREE)


def _build_weight_blob(weights, d):
    """Compact per-core weight blob: [32=(kk,i), (c,kc,j)=384, i'=4] fp16.

    The kernel zero-fills the [128, 6144] SBUF tile and DMAs this block into
    each batch's diagonal position (partition b*32+kk*4+i, column b*4+i').
    """
    wb = np.zeros((KCS, IB, C, KC, JB, IB), dtype=np.float32)
    w_dev = weights[IB * d : IB * d + IB]  # (IB, JB, C, K) -> i, j, c, k
    for i in range(IB):
        for kk in range(KCS):
            for kc in range(KC):
                # wb[kk, i, c, kc, j, i] = w_dev[i, j, c, kc*KCS+kk]
                wb[kk, i, :, kc, :, i] = w_dev[i, :, :, kc * KCS + kk].T
    return wb.reshape(32, C * KC * JB * IB).astype(_NPDT)


def kernel(x, weights):
    x = np.asarray(x, dtype=np.float32)
    weights = np.asarray(weights, dtype=np.float32)

    if "nc" not in _CACHE:
        _CACHE["nc"] = _build_program()
    nc = _CACHE["nc"]

    in_maps = []
    for d in range(NCORES):
        xs = _host_arrange_x(x[:, :, :, HD * d : HD * (d + 1), :])
        wbs = _build_weight_blob(weights, d)
        in_maps.append({"x": xs, "wb": wbs})

    res = run_bass_kernel_spmd(
        nc, in_maps, core_ids=list(range(NCORES)), **_CACHE.get("run_kwargs", {})
    )
    _CACHE["last_res"] = res
    # out staging [G=(b,i), (c,r,w)] per core -> (B, C, HD, W) -> concat H
    outs = []
    for d in range(NCORES):
        o = res.results[d]["out"].astype(np.float32).reshape(B, IB, C, BS, W)
        outs.append(o.transpose(0, 2, 1, 3, 4).reshape(B, C, HD, W))
    return np.concatenate(outs, axis=2)


# revision 31
# speedup vs baseline: 1.6570x; 1.6570x over previous
"""Blockwise linear fusion kernel for Trainium2 (8 NeuronCores).

Computes out[b,c,h,w] = sum_k x[b,k,c,h,w] * weights[h//16, w//16, c, k]
  x: (4, 32, 3, 512, 512) f32, weights: (32, 32, 3, 32) f32 -> out: (4, 3, 512, 512) f32

Strategy:
 - Shard H across the 8 cores: each core handles 64 rows = 4 row-blocks.
 - On each core, the K=32 weighted reduction runs on TensorE as block-diagonal
   matmuls: SBUF x-tiles are laid out [partition=(b,k8,i), free=(r16,w256)],
   and for each output 16x16 block a matmul with a [128,16] block-diagonal
   weight tile contracts k within 16 (b,i) groups -> out[16, 256] in PSUM,
   accumulated over 4 k-chunks.
 - The host pre-transposes each core's x slice (cast to fp16) into the exact
   tile layout so every x DMA is a flat contiguous [128, 8192] transfer, and
   pre-expands the weights into the block-diagonal SBUF layout.
 - Input tiles stream on the Sync HWDGE ring; weight/output DMAs ride the
   Scalar ring so their semaphore waits never stall the input stream.
"""

import sys

sys.path.insert(0, "/opt/trn_rl_repo")

import numpy as np

import concourse.bass as bass  # noqa: F401
import concourse.mybir as mybir
import concourse.tile as tile
from concourse import bacc
from concourse.bass_utils import run_bass_kernel_spmd

# Problem constants (hardcoded per harness contract)
B, K, C, H, W = 4, 32, 3, 512, 512
BS = 16
NCORES = 8
HD = H // NCORES  # 64 rows per core
IB = HD // BS  # 4 i-blocks per core
JB = W // BS  # 32 j-blocks
KC = 4  # number of k-chunks
KCS = K // KC  # 8 k per chunk
G = B * IB  # 16 groups (b, i)
WHALF = W // 2  # 256
JH = JB // 2  # 16 j's per w-half
TFREE = BS * WHALF  # 4096 free elements per (kc, w-half) chunk

_DT = mybir.dt.float16  # matmul input dtype (full-rate PE, half DMA traffic)
_NPDT = np.float16
_F32 = mybir.dt.float32

_CACHE = {}


class _FastEndTileContext(tile.TileContext):
    """TileContext with a cheaper epilogue: the stock one runs two full
    EVSEM butterfly barriers (~1.4us/hop via the DMA queue); sem-only
    barriers skip the per-engine InstDrains."""

    def _drain_and_barrier(self, tick_clock, wait_clock):
        from concourse.vector_clock import ScopedClock

        drain_inst = self.nc.sync.drain()
        wait_clock.add_sem_waits(
            drain_inst.ins, ScopedClock({None: tick_clock.global_clock})
        )
        self.nc.all_engine_barrier(sem_only=True)
        popped = self.nc._tile_sem_poison_stack.pop()
        assert popped is self._sem_poison
        self.nc.clear_and_free_semaphores(list(self.sems.allocated().values()))
        self.nc.all_engine_barrier(sem_only=True)


def _build_program():
    nc = bacc.Bacc(
        "TRN2",
        target_bir_lowering=False,
        debug=False,
        num_devices=NCORES,
        enable_partition_id=False,
    )

    # x pre-arranged on host: [c, wh, kcp, partition=(b,kk,i), free=(kc2,r,w')]
    x_d = nc.dram_tensor("x", [C, 2, KC // 2, 128, 2 * TFREE], _DT, kind="ExternalInput").ap()
    wb_d = nc.dram_tensor("wb", [128, C * KC * JB * G], _DT, kind="ExternalInput").ap()
    # out in staging layout: [partition=(b,i), free=(c,r,w)]; host un-permutes
    out_d = nc.dram_tensor("out", [G, C * BS * W], _DT, kind="ExternalOutput").ap()
    outv = out_d.rearrange("g (c r w) -> g c r w", c=C, r=BS)

    with _FastEndTileContext(nc) as tc:
        with (
            tc.tile_pool(name="wpool", bufs=1) as wpool,
            tc.tile_pool(name="xpool", bufs=6) as xpool,
            tc.tile_pool(name="opool", bufs=3) as opool,
            tc.tile_pool(name="ppool", bufs=8, space="PSUM") as ppool,
        ):
            wsb = wpool.tile([128, C * KC * JB * G], _DT)
            nc.gpsimd.dma_start(wsb[:], wb_d)

            for c in range(C):
                for wh in range(2):
                    last_round = c == C - 1 and wh == 1
                    # per-round output staging: [16=(b,i), free=(r,jl,q)]
                    osb = opool.tile([G, BS * JH * BS], _DT)
                    osbv = osb[:].rearrange("g (r j q) -> g j r q", r=BS, j=JH)
                    banks = [
                        ppool.tile([G, 512], _F32, name="bank", tag="bank")
                        for _ in range(8)
                    ]
                    for kcp in range(KC // 2):
                        if not last_round:
                            # one 2MB transfer covering two k-chunks
                            xt = xpool.tile([128, 2 * TFREE], _DT)
                            ring = (
                                nc.sync
                                if ((c * 2 + wh) * 2 + kcp) % 2 == 0
                                else nc.scalar
                            )
                            ring.dma_start(xt[:], x_d[c, wh, kcp])
                            xv = xt[:].rearrange(
                                "p (k2 r w) -> p k2 r w", k2=2, r=BS
                            )
                            subviews = [xv[:, 0], xv[:, 1]]
                        else:
                            # kc-granular 1MB transfers: later matmuls start
                            # sooner, shortening the kernel tail
                            subviews = []
                            for kc2 in range(2):
                                xts = xpool.tile(
                                    [128, TFREE], _DT, name="xts", tag="xts", bufs=4
                                )
                                ring = nc.sync if kc2 == 0 else nc.scalar
                                ring.dma_start(
                                    xts[:],
                                    x_d[c, wh, kcp][
                                        :, kc2 * TFREE : (kc2 + 1) * TFREE
                                    ],
                                )
                                subviews.append(
                                    xts[:].rearrange("p (r w) -> p r w", r=BS)
                                )
                        for kc2 in range(2):
                            kc = kcp * 2 + kc2
                            xvk = subviews[kc2]
                            for jl in range(JH):
                                j = wh * JH + jl  # global j block
                                m = jl // 2  # bank index
                                half = jl % 2
                                col0 = ((c * KC + kc) * JB + j) * G
                                nc.tensor.matmul(
                                    banks[m][:, half * 256 : half * 256 + 256],
                                    wsb[:, col0 : col0 + G],
                                    xvk[:, :, jl * BS : (jl + 1) * BS],
                                    start=(kc == 0 and half == 0),
                                    stop=(kc == KC - 1 and half == 1),
                                )
                    # evacuate psum -> osb; stream out on the SWDGE queue so
                    # store waits never stall the input rings
                    for m in range(8):
                        srcv = banks[m][:].rearrange(
                            "g (jj r q) -> g jj r q", jj=2, r=BS, q=BS
                        )
                        nc.vector.tensor_copy(osbv[:, 2 * m : 2 * m + 2, :, :], srcv)
                        if last_round and m % 2 == 1:
                            # pair-granular stores shorten the kernel tail
                            w0 = wh * WHALF + (m - 1) * 2 * BS
                            ow = outv[:, c, :, w0 : w0 + 4 * BS]
                            osl = osb[:].rearrange("g (r w) -> g r w", r=BS)[
                                :, :, (m - 1) * 2 * BS : (m + 1) * 2 * BS
                            ]
                            nc.gpsimd.dma_start(ow, osl)
                    if not last_round:
                        ow = outv[:, c, :, wh * WHALF : (wh + 1) * WHALF]
                        osl = osb[:].rearrange("g (r w) -> g r w", r=BS)
                        nc.gpsimd.dma_start(ow, osl)

    nc.compile()
    return nc


def _host_arrange_x(x_dev):
    """(B, K, C, HD, W) -> [C, 2, KC/2, 128, 2*TFREE] fp16 tile layout.

    partition p = b*(KCS*IB) + kk*IB + i ; free f = kc2*TFREE + r*WHALF + w'
    """
    t = x_dev.reshape(B, KC // 2, 2, KCS, C, IB, BS, 2, WHALF)
    # -> c, wh, kcp, b, kk, i, kc2, r, w'
    t = t.transpose(4, 7, 1, 0, 3, 5, 2, 6, 8)
    return t.astype(_NPDT).reshape(C, 2, KC // 2, 128, 2 * TFREE)


def _build_weight_blob(weights, d):
    """Block-diagonal weight layout for core d: [128, C*KC*JB*G] fp16."""
    wb = np.zeros((128, C, KC, JB, G), dtype=np.float32)
    # partition p = b*32 + kk*4 + i ; col g' = b*4 + i
    w_dev = weights[IB * d : IB * d + IB]  # (IB, JB, C, K) -> i, j, c, k
    for b in range(B):
        for i in range(IB):
            g = b * IB + i
            for kk in range(KCS):
                p = b * (KCS * IB) + kk * IB + i
                for kc in range(KC):
                    # wb[p, c, kc, j, g] = w_dev[i, j, c, kc*KCS+kk]
                    wb[p, :, kc, :, g] = w_dev[i, :, :, kc * KCS + kk].T
    return wb.reshape(128, C * KC * JB * G).astype(_NPDT)


def kernel(x, weights):
    x = np.asarray(x, dtype=np.float32)
    weights = np.asarray(weights, dtype=np.float32)

    if "nc" not in _CACHE:
        _CACHE["nc"] = _build_program()
    nc = _CACHE["nc"]

    in_maps = []
    for d in range(NCORES):
        xs = _host_arrange_x(x[:, :, :, HD * d : HD * (d + 1), :])
        wbs = _build_weight_blob(weights, d)
        in_maps.append({"x": xs, "wb": wbs})

    res = run_bass_kernel_spmd(
        nc, in_maps, core_ids=list(range(NCORES)), **_CACHE.get("run_kwargs", {})
    )
    _CACHE["last_res"] = res
    # out staging [G=(b,i), (c,r,w)] per core -> (B, C, HD, W) -> concat H
    outs = []
    for d in range(NCORES):
        o = res.results[d]["out"].astype(np.float32).reshape(B, IB, C, BS, W)
        outs.append(o.transpose(0, 2, 1, 3, 4).reshape(B, C, HD, W))
    return np.concatenate(outs, axis=2)
